# revision 68
# baseline (speedup 1.0000x reference)
"""AffinityLoss (segment-reduce) Trainium2 kernel.

Math (single pass over the data — no per-row center gather needed):
    lbl     = argmax(labels, axis=1)                         (N,)
    sums_c  = sum of features rows with lbl == c             (C, D)
    n_c     = count of rows with lbl == c                    (C,)
    sumsq   = sum(features ** 2)                             scalar
    centers = where(n>0, sums/max(n,1), 0) + 1e-6
    intra   = sumsq - 2*sum(sums*centers) + sum(n_c*||c_c||^2)
    inter   = sum((centers - mean(centers))^2) / C
    loss    = intra / (inter + 1e-6)

Per core (data-parallel over N):
  - one-hot(argmax) built on the vector engine (reduce_max + one
    broadcast is_equal over the whole supertile)
  - segment sums via PE: one matmul per 128-row group
    (one-hot^T @ features) accumulated in PSUM over the full loop
  - counts via PE with ones as the stationary operand; ALL counts
    matmuls (every j-group of every supertile) accumulate into a single
    [1, cc*C] PSUM tile (accumulation commutes), which closes at the
    last MAIN supertile so the counts copy + DMA-out happen mid-stream
  - sum-of-squares on the scalar engine (Square activation + accumulate)

Features stream as f32 -> bf16 cast DMAs (SWDGE), contiguous per
partition per supertile; the supertile schedule tapers at the end so the
compute tail after the last DMA is short.  ALL label DMAs are issued up
front into distinct resident SBUF tiles (~100KB/partition), split across
BOTH the sync and scalar HWDGE rings: the DMA engines round-robin across
descriptor rings, so two label rings get ~2x the ring share of the
feature stream and the whole label stream lands by ~2/3 of the kernel.
One-hots are built on the DVE a few supertiles ahead of the features,
so after the final feature DMA only the last sums matmuls + the [C,D]
PSUM readout remain.  Keeping the in-loop sync/scalar instruction
streams free of DMA issues matters: a mid-loop blocked issue stalls
squares, then feature-buffer recycling, then the whole stream.  The
readout is split in two halves whose DMA issues run on the sync and
scalar sequencers in parallel; the main sqacc columns ship mid-stream
so only the tail columns (gated by the scalar square backlog) remain at
the end.  The O(C*D) finalization runs on the host over the 8 per-core
partials (the gather/unshard step).

Measured: ~130.5us HW exec on 8 cores at full clock (baseline 132.9us).
Roofline: 46.66MB/core over 16 DMA queues at ~26GB/s each ≈ 111.5us of
pure wire time; ~8us of fixed preamble + SWDGE boot; ~4us tail+drain.
"""

import numpy as np

import concourse.bacc as bacc
import concourse.tile as tile
from concourse import mybir
from concourse.bass_utils import run_bass_kernel_spmd

N_CORES = 8
N_TOTAL = 262144
D = 256
C = 100
P = 128
T = 16  # 128-row groups per supertile (DMA batch)

F32 = mybir.dt.float32
BF16 = mybir.dt.bfloat16


def build_nc(
    rows_per_core: int,
    t: int = T,
    bufs: int = 6,
):
    """Build the per-core Bass program (same SPMD program on all cores)."""
    total_j = rows_per_core // P
    cc = 4  # j's per counts matmul (free dim cc*C <= 512)
    assert t % cc == 0
    # Supertile schedule: mostly t, tapering at the end so the compute tail
    # after the final DMA is short.
    if t % 8 == 0:
        tail = [t // 2, t // 4, t // 8, t // 8]
    else:
        tail = [t // 2, t // 2]
    if total_j > 2 * t and t >= 8 and (total_j - sum(tail)) % t == 0:
        sched = [t] * ((total_j - sum(tail)) // t) + tail
        n_main = len(sched) - len(tail)
    else:
        assert total_j % t == 0
        sched = [t] * (total_j // t)
        n_main = len(sched)
    assert sum(sched) == total_j
    n_super = len(sched)


    nc = bacc.Bacc(
        "TRN2", target_bir_lowering=False, debug=False, num_devices=N_CORES
    )

    feats = nc.dram_tensor(
        "features", [rows_per_core, D], F32, kind="ExternalInput"
    ).ap()
    labels = nc.dram_tensor(
        "labels", [rows_per_core, C], F32, kind="ExternalInput"
    ).ap()
    out_partial = nc.dram_tensor(
        "partial", [C, D], F32, kind="ExternalOutput"
    ).ap()
    out_counts = nc.dram_tensor(
        "counts", [1, cc * C], F32, kind="ExternalOutput"
    ).ap()
    have_tail = n_main < n_super
    out_sqacc = nc.dram_tensor(
        "sqacc", [P, n_super], F32, kind="ExternalOutput"
    ).ap()

    # Blocked row mapping per supertile: row = row0 + p*ts + j -> partition p
    # reads ts contiguous rows (one contiguous DRAM chunk per partition).

    lead = 3  # one-hot (DVE) lead over the feature stream, in supertiles

    with tile.TileContext(nc) as tc:
        with (
            tc.tile_pool(name="feat", bufs=bufs) as feat_pool,
            tc.tile_pool(name="oh", bufs=lead + 2) as oh_pool,
            tc.tile_pool(name="sq", bufs=2) as sq_pool,
            tc.tile_pool(name="acc", bufs=1) as acc_pool,
            tc.tile_pool(name="ps", bufs=1, space="PSUM") as psum_pool,
        ):
            psum_sums = psum_pool.tile([C, D], F32, tag="ps_sums")
            psum_cnt = psum_pool.tile([1, cc * C], F32, tag="ps_cnt")
            sqacc = acc_pool.tile([P, n_super], F32, tag="sqacc")
            ones = acc_pool.tile([P, 1], BF16, tag="ones")
            part_sb = acc_pool.tile([C, D], F32, tag="part")
            cnt_sb = acc_pool.tile([1, cc * C], F32, tag="cnt")
            nc.vector.memset(ones[:, :], 1.0)

            def make_onehot(lbl_ap, mx, oh, ts):
                nc.vector.reduce_max(
                    mx[:, :ts], lbl_ap, axis=mybir.AxisListType.X
                )
                mxb = mx[:, :ts].unsqueeze(-1).broadcast_to((P, ts, C))
                nc.vector.tensor_tensor(
                    out=oh[:, :ts, :], in0=lbl_ap, in1=mxb,
                    op=mybir.AluOpType.is_equal,
                )

            def cnt_matmul(oh, j0, w, start, stop):
                # ones^T @ onehot[:, j0:j0+w] -> per-(j,c) column counts,
                # accumulated into the single shared counts PSUM region.
                nc.tensor.matmul(
                    psum_cnt[:, : w * C],
                    ones[:, :],
                    oh[:, j0 : j0 + w],
                    start=start,
                    stop=stop,
                )

            row_start = [P * sum(sched[:i]) for i in range(n_super)]
            onehots = {}

            # ALL label DMAs are issued up front into distinct resident
            # tiles (first-uses only, so no WAR waits can ever block a
            # sequencer mid-loop).  The DMA engines round-robin across
            # descriptor rings, so splitting the issues over TWO HWDGE
            # rings (sync + scalar) gives labels ~2x the ring share of
            # the SWDGE feature stream: the whole label stream lands by
            # ~2/3 of the kernel and every one-hot is ready early, while
            # the in-loop sync/scalar instruction streams stay free of
            # DMA issues (a mid-loop blocked issue would stall squares,
            # feature-buffer recycling and eventually the stream).
            lbl_tiles = []
            for i in range(n_super):
                ts = sched[i]
                r0 = row_start[i]
                lv = labels[r0 : r0 + P * ts].rearrange(
                    "(p j) c -> p j c", p=P, j=ts
                )
                lt = acc_pool.tile(
                    [P, ts, C], F32, tag=f"lbl{i}", name=f"lbl{i}"
                )
                eng = nc.sync if i % 2 == 0 else nc.scalar
                eng.dma_start(out=lt[:, :, :], in_=lv)
                lbl_tiles.append(lt)

            def emit_onehot(i):
                # one-hot for supertile i on the DVE, `lead` supertiles
                # before its features arrive (labels land far earlier)
                ts = sched[i]
                mx = oh_pool.tile([P, t], F32, tag="mx")
                oh = oh_pool.tile([P, t, C], BF16, tag="oh")
                make_onehot(lbl_tiles[i][:, :, :], mx, oh, ts)
                onehots[i] = oh

            for i in range(min(lead, n_super)):
                emit_onehot(i)

            for s, ts in enumerate(sched):
                fv = feats[row_start[s] : row_start[s] + P * ts].rearrange(
                    "(p j) d -> p j d", p=P, j=ts
                )
                feat_t = feat_pool.tile([P, t, D], BF16, tag="feat")
                # SWDGE (gpsimd) casts f32 -> bf16 during the transfer
                nc.gpsimd.dma_start(out=feat_t[:, :ts, :], in_=fv)

                if s + lead < n_super:
                    emit_onehot(s + lead)
                onehot = onehots.pop(s)

                sq_t = sq_pool.tile([P, t, D], BF16, tag="sq")
                nc.scalar.activation(
                    sq_t[:, :ts, :],
                    feat_t[:, :ts, :],
                    mybir.ActivationFunctionType.Square,
                    accum_out=sqacc[:, s : s + 1],
                )
                # main sqacc columns are complete once sq(n_main-1) ran:
                # ship them mid-stream on the (idle) sync sequencer so only
                # the tiny tail columns remain at the end.
                if have_tail and s == n_main - 1:
                    nc.sync.dma_start(
                        out=out_sqacc[:, :n_main], in_=sqacc[:, :n_main]
                    )

                for j in range(ts):
                    nc.tensor.matmul(
                        psum_sums[:, :],
                        onehot[:, j],
                        feat_t[:, j],
                        start=(s == 0 and j == 0),
                        stop=(s == n_super - 1 and j == ts - 1),
                    )
                # counts inline for every supertile; the one-hot is always
                # ready early, so the last tail group closes the shared
                # PSUM right behind the final sums matmuls.
                for g in range(0, ts, cc):
                    w = min(cc, ts - g)
                    cnt_matmul(
                        onehot,
                        g,
                        w,
                        start=(s == 0 and g == 0),
                        stop=(s == n_super - 1 and g + cc >= ts),
                    )

            # Final readout: partial halves first (critical path), copies
            # on vector, DMA issues spread over sync + scalar in parallel;
            # then counts (sync) and sqacc (gpsimd, whose producer chain --
            # pure Squares on scalar -- is never blocked).
            h = D // 2
            nc.vector.tensor_copy(part_sb[:, :h], psum_sums[:, :h])
            nc.sync.dma_start(out=out_partial[:, :h], in_=part_sb[:, :h])
            nc.vector.tensor_copy(part_sb[:, h:], psum_sums[:, h:])
            nc.scalar.dma_start(out=out_partial[:, h:], in_=part_sb[:, h:])
            nc.vector.tensor_copy(cnt_sb[:, :], psum_cnt[:, :])
            nc.sync.dma_start(out=out_counts[:, :], in_=cnt_sb[:, :])
            # keep the gpsimd stream pure feature issues: a scheduler draw
            # that hoists a data-gated epilogue issue above the last
            # feature issues would stall the stream
            if have_tail:
                nc.sync.dma_start(
                    out=out_sqacc[:, n_main:], in_=sqacc[:, n_main:]
                )
            else:
                nc.sync.dma_start(out=out_sqacc[:, :], in_=sqacc[:, :])

    nc.compile()
    return nc


_NC_CACHE: dict = {}


def _get_nc():
    if "nc" not in _NC_CACHE:
        _NC_CACHE["nc"] = build_nc(N_TOTAL // N_CORES)
    return _NC_CACHE["nc"]


def finalize(partials, countss, sqaccs):
    """Host gather/unshard: combine per-core partials into the scalar loss."""
    sums = np.zeros((C, D), np.float64)
    counts = np.zeros((C,), np.float64)
    sumsq = 0.0
    for part, cnt, sq in zip(partials, countss, sqaccs):
        sums += part.astype(np.float64).reshape(C, -1, D).sum(axis=1)
        counts += cnt.astype(np.float64).reshape(-1, C).sum(axis=0)
        sumsq += float(sq.astype(np.float64).sum())
    centers = (
        np.where(counts[:, None] > 0, sums / np.maximum(counts, 1.0)[:, None], 0.0)
        + 1e-6
    )
    intra = (
        sumsq
        - 2.0 * float((sums * centers).sum())
        + float((counts * (centers**2).sum(axis=1)).sum())
    )
    cmean = centers.mean(axis=0, keepdims=True)
    inter = float(((centers - cmean) ** 2).sum()) / C
    loss = intra / (inter + 1e-6)
    return np.array(loss, dtype=np.float32)


def kernel(features: np.ndarray, labels: np.ndarray) -> np.ndarray:
    features = np.asarray(features)
    labels = np.asarray(labels)
    assert features.shape == (N_TOTAL, D), features.shape
    assert labels.shape == (N_TOTAL, C), labels.shape
    nc = _get_nc()
    rows = N_TOTAL // N_CORES
    in_maps = []
    for i in range(N_CORES):
        sl = slice(i * rows, (i + 1) * rows)
        in_maps.append(
            {
                "features": np.ascontiguousarray(features[sl], dtype=np.float32),
                "labels": np.ascontiguousarray(labels[sl], dtype=np.float32),
            }
        )
    res = run_bass_kernel_spmd(nc, in_maps, list(range(N_CORES)))
    return finalize(
        [r["partial"] for r in res.results],
        [r["counts"] for r in res.results],
        [r["sqacc"] for r in res.results],
    )


# revision 69
# speedup vs baseline: 1.1994x; 1.1994x over previous
"""AffinityLoss (segment-reduce) Trainium2 kernel.

Math (single pass over the data — no per-row center gather needed):
    lbl     = argmax(labels, axis=1)                         (N,)
    sums_c  = sum of features rows with lbl == c             (C, D)
    n_c     = count of rows with lbl == c                    (C,)
    sumsq   = sum(features ** 2)                             scalar
    centers = where(n>0, sums/max(n,1), 0) + 1e-6
    intra   = sumsq - 2*sum(sums*centers) + sum(n_c*||c_c||^2)
    inter   = sum((centers - mean(centers))^2) / C
    loss    = intra / (inter + 1e-6)

Per core (data-parallel over N):
  - one-hot(argmax) built on the vector engine (reduce_max + one
    broadcast is_equal over the whole supertile)
  - segment sums via PE: one matmul per 128-row group
    (one-hot^T @ features) accumulated in PSUM over the full loop
  - counts via PE with ones as the stationary operand; ALL counts
    matmuls (every j-group of every supertile) accumulate into a single
    [1, cc*C] PSUM tile (accumulation commutes), which closes at the
    last MAIN supertile so the counts copy + DMA-out happen mid-stream
  - sum-of-squares on the scalar engine (Square activation + accumulate)

Features stream as f32 -> bf16 cast DMAs (SWDGE), contiguous per
partition per supertile; the supertile schedule tapers at the end so the
compute tail after the last DMA is short.  ALL label DMAs are issued up
front into distinct resident SBUF tiles (~100KB/partition), split across
BOTH the sync and scalar HWDGE rings: the DMA engines round-robin across
descriptor rings, so two label rings get ~2x the ring share of the
feature stream and the whole label stream lands by ~2/3 of the kernel.
One-hots are built on the DVE a few supertiles ahead of the features,
so after the final feature DMA only the last sums matmuls + the [C,D]
PSUM readout remain.  Keeping the in-loop sync/scalar instruction
streams free of DMA issues matters: a mid-loop blocked issue stalls
squares, then feature-buffer recycling, then the whole stream.  The
readout is split in two halves whose DMA issues run on the sync and
scalar sequencers in parallel; the main sqacc columns ship mid-stream
so only the tail columns (gated by the scalar square backlog) remain at
the end.  The O(C*D) finalization runs on the host over the 8 per-core
partials (the gather/unshard step).

Measured: ~130.5us HW exec on 8 cores at full clock (baseline 132.9us).
Roofline: 46.66MB/core over 16 DMA queues at ~26GB/s each ≈ 111.5us of
pure wire time; ~8us of fixed preamble + SWDGE boot; ~4us tail+drain.
"""

import numpy as np

import concourse.bacc as bacc
import concourse.tile as tile
from concourse import mybir
from concourse.bass_utils import run_bass_kernel_spmd

N_CORES = 8
N_TOTAL = 262144
D = 256
C = 100
P = 128
T = 16  # 128-row groups per supertile (DMA batch)

F32 = mybir.dt.float32
BF16 = mybir.dt.bfloat16


def build_nc(
    rows_per_core: int,
    t: int = T,
    bufs: int = 6,
):
    """Build the per-core Bass program (same SPMD program on all cores)."""
    total_j = rows_per_core // P
    cc = 4  # j's per counts matmul (free dim cc*C <= 512)
    assert t % cc == 0
    # Supertile schedule: mostly t, tapering at the end so the compute tail
    # after the final DMA is short.
    if t % 8 == 0:
        tail = [t // 2, t // 4, t // 8, t // 8]
    else:
        tail = [t // 2, t // 2]
    if total_j > 2 * t and t >= 8 and (total_j - sum(tail)) % t == 0:
        sched = [t] * ((total_j - sum(tail)) // t) + tail
        n_main = len(sched) - len(tail)
    else:
        assert total_j % t == 0
        sched = [t] * (total_j // t)
        n_main = len(sched)
    assert sum(sched) == total_j
    n_super = len(sched)


    nc = bacc.Bacc(
        "TRN2", target_bir_lowering=False, debug=False, num_devices=N_CORES
    )

    feats = nc.dram_tensor(
        "features", [rows_per_core, D], F32, kind="ExternalInput"
    ).ap()
    labels = nc.dram_tensor(
        "labels", [rows_per_core, C], F32, kind="ExternalInput"
    ).ap()
    out_partial = nc.dram_tensor(
        "partial", [C, D], F32, kind="ExternalOutput"
    ).ap()
    out_counts = nc.dram_tensor(
        "counts", [1, cc * C], F32, kind="ExternalOutput"
    ).ap()
    have_tail = n_main < n_super
    out_sqacc = nc.dram_tensor(
        "sqacc", [P, n_super], F32, kind="ExternalOutput"
    ).ap()

    # Blocked row mapping per supertile: row = row0 + p*ts + j -> partition p
    # reads ts contiguous rows (one contiguous DRAM chunk per partition).

    lead = 3  # one-hot (DVE) lead over the feature stream, in supertiles

    with tile.TileContext(nc) as tc:
        with (
            tc.tile_pool(name="feat", bufs=bufs) as feat_pool,
            tc.tile_pool(name="oh", bufs=lead + 2) as oh_pool,
            tc.tile_pool(name="sq", bufs=2) as sq_pool,
            tc.tile_pool(name="acc", bufs=1) as acc_pool,
            tc.tile_pool(name="ps", bufs=1, space="PSUM") as psum_pool,
        ):
            psum_sums = psum_pool.tile([C, D], F32, tag="ps_sums")
            psum_cnt = psum_pool.tile([1, cc * C], F32, tag="ps_cnt")
            sqacc = acc_pool.tile([P, n_super], F32, tag="sqacc")
            ones = acc_pool.tile([P, 1], BF16, tag="ones")
            part_sb = acc_pool.tile([C, D], F32, tag="part")
            cnt_sb = acc_pool.tile([1, cc * C], F32, tag="cnt")
            nc.vector.memset(ones[:, :], 1.0)

            def make_onehot(lbl_ap, mx, oh, ts):
                nc.vector.reduce_max(
                    mx[:, :ts], lbl_ap, axis=mybir.AxisListType.X
                )
                mxb = mx[:, :ts].unsqueeze(-1).broadcast_to((P, ts, C))
                nc.vector.tensor_tensor(
                    out=oh[:, :ts, :], in0=lbl_ap, in1=mxb,
                    op=mybir.AluOpType.is_equal,
                )

            def cnt_matmul(oh, j0, w, start, stop):
                # ones^T @ onehot[:, j0:j0+w] -> per-(j,c) column counts,
                # accumulated into the single shared counts PSUM region.
                nc.tensor.matmul(
                    psum_cnt[:, : w * C],
                    ones[:, :],
                    oh[:, j0 : j0 + w],
                    start=start,
                    stop=stop,
                )

            row_start = [P * sum(sched[:i]) for i in range(n_super)]
            onehots = {}

            # ALL label DMAs are issued up front into distinct resident
            # tiles (first-uses only, so no WAR waits can ever block a
            # sequencer mid-loop).  The DMA engines round-robin across
            # descriptor rings, so splitting the issues over TWO HWDGE
            # rings (sync + scalar) gives labels ~2x the ring share of
            # the SWDGE feature stream: the whole label stream lands by
            # ~2/3 of the kernel and every one-hot is ready early, while
            # the in-loop sync/scalar instruction streams stay free of
            # DMA issues (a mid-loop blocked issue would stall squares,
            # feature-buffer recycling and eventually the stream).
            lbl_tiles = []
            for i in range(n_super):
                ts = sched[i]
                r0 = row_start[i]
                lv = labels[r0 : r0 + P * ts].rearrange(
                    "(p j) c -> p j c", p=P, j=ts
                )
                lt = acc_pool.tile(
                    [P, ts, C], F32, tag=f"lbl{i}", name=f"lbl{i}"
                )
                eng = nc.sync if i % 2 == 0 else nc.scalar
                eng.dma_start(out=lt[:, :, :], in_=lv)
                lbl_tiles.append(lt)

            def emit_onehot(i):
                # one-hot for supertile i on the DVE, `lead` supertiles
                # before its features arrive (labels land far earlier)
                ts = sched[i]
                mx = oh_pool.tile([P, t], F32, tag="mx")
                oh = oh_pool.tile([P, t, C], BF16, tag="oh")
                make_onehot(lbl_tiles[i][:, :, :], mx, oh, ts)
                onehots[i] = oh

            for i in range(min(lead, n_super)):
                emit_onehot(i)

            for s, ts in enumerate(sched):
                fv = feats[row_start[s] : row_start[s] + P * ts].rearrange(
                    "(p j) d -> p j d", p=P, j=ts
                )
                feat_t = feat_pool.tile([P, t, D], BF16, tag="feat")
                # SWDGE (gpsimd) casts f32 -> bf16 during the transfer
                nc.gpsimd.dma_start(out=feat_t[:, :ts, :], in_=fv)

                if s + lead < n_super:
                    emit_onehot(s + lead)
                onehot = onehots.pop(s)

                sq_t = sq_pool.tile([P, t, D], BF16, tag="sq")
                nc.scalar.activation(
                    sq_t[:, :ts, :],
                    feat_t[:, :ts, :],
                    mybir.ActivationFunctionType.Square,
                    accum_out=sqacc[:, s : s + 1],
                )
                # main sqacc columns are complete once sq(n_main-1) ran:
                # ship them mid-stream on the (idle) sync sequencer so only
                # the tiny tail columns remain at the end.
                if have_tail and s == n_main - 1:
                    nc.sync.dma_start(
                        out=out_sqacc[:, :n_main], in_=sqacc[:, :n_main]
                    )

                for j in range(ts):
                    nc.tensor.matmul(
                        psum_sums[:, :],
                        onehot[:, j],
                        feat_t[:, j],
                        start=(s == 0 and j == 0),
                        stop=(s == n_super - 1 and j == ts - 1),
                    )
                # counts inline for every supertile; the one-hot is always
                # ready early, so the last tail group closes the shared
                # PSUM right behind the final sums matmuls.
                for g in range(0, ts, cc):
                    w = min(cc, ts - g)
                    cnt_matmul(
                        onehot,
                        g,
                        w,
                        start=(s == 0 and g == 0),
                        stop=(s == n_super - 1 and g + cc >= ts),
                    )

            # Final readout: partial halves first (critical path), copies
            # on vector, DMA issues spread over sync + scalar in parallel;
            # then counts (sync) and sqacc (gpsimd, whose producer chain --
            # pure Squares on scalar -- is never blocked).
            h = D // 2
            nc.vector.tensor_copy(part_sb[:, :h], psum_sums[:, :h])
            nc.sync.dma_start(out=out_partial[:, :h], in_=part_sb[:, :h])
            nc.vector.tensor_copy(part_sb[:, h:], psum_sums[:, h:])
            nc.scalar.dma_start(out=out_partial[:, h:], in_=part_sb[:, h:])
            if have_tail:
                nc.gpsimd.dma_start(
                    out=out_sqacc[:, n_main:], in_=sqacc[:, n_main:]
                )
            else:
                nc.gpsimd.dma_start(out=out_sqacc[:, :], in_=sqacc[:, :])
            nc.vector.tensor_copy(cnt_sb[:, :], psum_cnt[:, :])
            nc.gpsimd.dma_start(out=out_counts[:, :], in_=cnt_sb[:, :])

    nc.compile()
    return nc


_NC_CACHE: dict = {}


def _get_nc():
    if "nc" not in _NC_CACHE:
        _NC_CACHE["nc"] = build_nc(N_TOTAL // N_CORES)
    return _NC_CACHE["nc"]


def finalize(partials, countss, sqaccs):
    """Host gather/unshard: combine per-core partials into the scalar loss."""
    sums = np.zeros((C, D), np.float64)
    counts = np.zeros((C,), np.float64)
    sumsq = 0.0
    for part, cnt, sq in zip(partials, countss, sqaccs):
        sums += part.astype(np.float64).reshape(C, -1, D).sum(axis=1)
        counts += cnt.astype(np.float64).reshape(-1, C).sum(axis=0)
        sumsq += float(sq.astype(np.float64).sum())
    centers = (
        np.where(counts[:, None] > 0, sums / np.maximum(counts, 1.0)[:, None], 0.0)
        + 1e-6
    )
    intra = (
        sumsq
        - 2.0 * float((sums * centers).sum())
        + float((counts * (centers**2).sum(axis=1)).sum())
    )
    cmean = centers.mean(axis=0, keepdims=True)
    inter = float(((centers - cmean) ** 2).sum()) / C
    loss = intra / (inter + 1e-6)
    return np.array(loss, dtype=np.float32)


def kernel(features: np.ndarray, labels: np.ndarray) -> np.ndarray:
    features = np.asarray(features)
    labels = np.asarray(labels)
    assert features.shape == (N_TOTAL, D), features.shape
    assert labels.shape == (N_TOTAL, C), labels.shape
    nc = _get_nc()
    rows = N_TOTAL // N_CORES
    in_maps = []
    for i in range(N_CORES):
        sl = slice(i * rows, (i + 1) * rows)
        in_maps.append(
            {
                "features": np.ascontiguousarray(features[sl], dtype=np.float32),
                "labels": np.ascontiguousarray(labels[sl], dtype=np.float32),
            }
        )
    res = run_bass_kernel_spmd(nc, in_maps, list(range(N_CORES)))
    return finalize(
        [r["partial"] for r in res.results],
        [r["counts"] for r in res.results],
        [r["sqacc"] for r in res.results],
    )
